# revision 1
# baseline (speedup 1.0000x reference)
"""Trainium2 Bass kernel for a two-window sparse causal self-attention block.

Model (B=2, T=2048, C=1024):
  - 8 "short" heads: d_qk=32,  window 256
  - 8 "long"  heads: d_qk=128, window 1024
  - value/output head dim 64, output projection C x C.

Sharding (8 cores): data-parallel over batch (2) x head-parallel over head
groups (4). Core c = 4*b + g handles batch b and heads {2g, 2g+1} of both the
short and long sets. Each core computes its 4 heads' attention plus the
corresponding 256 rows of Wproj, producing a partial [T, C] output; the host
sums the 4 partials per batch element.

Device-side design notes:
  - float32r matmuls everywhere: full PE rate (1 cycle/row at N>=256) vs 2
    cycles/row for fp32, ~1.5e-4 matmul relative error.
  - everything is computed in "transposed" orientation so no on-device
    transposes are needed: host passes xT [C, T]; projections give qT/kT
    [d, T] and v [T, dv]; scores sT[k, q] = kT.T @ qT; yT[dv, q] = v_aug.T @
    pT with a ones column in v so row 64 of yT accumulates softmax sums.
  - queries processed in groups of 512 (4 blocks) so score/AV matmuls run at
    N=512; the causal band mask is applied multiplicatively on exp(scores)
    using 512-wide sliding windows into a host-precomputed [128, W+896] band
    image.
  - exp skips the max-subtraction: inputs are well-scaled (|scores| < ~10).
  - normalization: reciprocal of the sums row, broadcast across partitions
    via a rank-1 matmul against a ones row, multiply into the yT tiles.
"""

import math

import numpy as np

import concourse.bass as bass
import concourse.mybir as mybir
import concourse.tile as tile
from concourse.bass_utils import run_bass_kernel_spmd

F32 = mybir.dt.float32
F32R = mybir.dt.float32r

B, T, C = 2, 2048, 1024
HS, DS = 8, 32
HL, DL = 8, 128
HD = 64
WIN_S, WIN_L = 256, 1024
NT = T // 128    # 16 t-blocks
NCB = C // 128   # 8 c-blocks
NG = T // 512    # 4 query groups
VW = HD + 1      # v columns + ones column for softmax sums
N_CORES = 8


def _split_waits(nc: bass.Bass) -> int:
    """Walrus in this env accepts at most 1 sync wait per instruction.
    Hoist extra waits onto same-engine InstNoOp instructions placed just
    before the owning instruction (same-engine program order preserves the
    blocking semantics)."""
    import bass_rust

    n_added = 0
    for f in nc.m.functions:
        for bb in f.blocks:
            insts = bb.instructions
            if not any(inst.sync_info and len(inst.sync_info.on_wait) > 1
                       for inst in insts):
                continue
            new = []
            for inst in insts:
                si = inst.sync_info
                waits = list(si.on_wait) if si else []
                if len(waits) > 1:
                    for i, w in enumerate(waits[:-1]):
                        nop = mybir.InstNoOp(
                            name=f"{inst.name}_hw{i}",
                            sync_info=bass_rust.SyncInfo(on_wait=[w], on_update=[]),
                            bass_nofuse=True,
                            engine=inst.engine,
                        )
                        new.append(nop)
                        n_added += 1
                    inst.sync_info = bass_rust.SyncInfo(
                        on_wait=waits[-1:], on_update=list(si.on_update))
                new.append(inst)
            bb.instructions = new
    return n_added


def _patch_tile_drain():
    """This walrus build rejects >1 sync wait on the TileContext tail drain
    ("Too many sync wait commands"). Re-emit the drain's waits as individual
    wait_ge instructions on the sync engine."""
    import bass_rust
    from concourse.tile import ScopedClock, TileContext

    def _drain_and_barrier(self, tick_clock, wait_clock):
        nc = self.nc
        drain_inst = nc.sync.drain()
        wait_clock.add_sem_waits(
            drain_inst.ins, ScopedClock({None: tick_clock.global_clock})
        )
        si = drain_inst.ins.sync_info
        waits = list(si.on_wait) if si is not None else []
        if len(waits) > 1:
            drain_inst.ins.sync_info = bass_rust.SyncInfo(on_wait=[], on_update=[])
            sems = {h.name: h for h in self.sems.allocated().values()}
            for w in waits:
                nc.sync.wait_ge(sems[w.ant_name], w.wait_value)
        nc.all_engine_barrier()
        popped = nc._tile_sem_poison_stack.pop()
        assert popped is self._sem_poison
        nc.clear_and_free_semaphores(list(self.sems.allocated().values()))
        nc.all_engine_barrier()

    TileContext._drain_and_barrier = _drain_and_barrier


_patch_tile_drain()


def _build_program() -> bass.Bass:
    nc = bass.Bass()

    xt_d = nc.dram_tensor("xt", [C, T], F32, kind="ExternalInput")
    wsqk_d = nc.dram_tensor("wsqk", [C, 128], F32, kind="ExternalInput")
    wql_d = nc.dram_tensor("wql", [C, 256], F32, kind="ExternalInput")
    wkl_d = nc.dram_tensor("wkl", [C, 256], F32, kind="ExternalInput")
    wv_d = nc.dram_tensor("wv", [C, 256], F32, kind="ExternalInput")
    wp_d = nc.dram_tensor("wp", [256, C], F32, kind="ExternalInput")
    bs_d = nc.dram_tensor("band_s", [128, WIN_S + 896], F32, kind="ExternalInput")
    bl_d = nc.dram_tensor("band_l", [128, WIN_L + 896], F32, kind="ExternalInput")
    ones_d = nc.dram_tensor("ones", [128, 64], F32, kind="ExternalInput")
    out_d = nc.dram_tensor("out", [T, C], F32, kind="ExternalOutput")

    scale_s = 1.0 / math.sqrt(DS)
    scale_l = 1.0 / math.sqrt(DL)

    with tile.TileContext(nc) as tc:
        with (
            tc.tile_pool(name="const", bufs=1) as const,
            tc.tile_pool(name="qkp", bufs=1) as qkp,
            tc.tile_pool(name="vp", bufs=1) as vp,
            tc.tile_pool(name="bigps", bufs=2, space="PSUM") as bigps,
            tc.tile_pool(name="yhps", bufs=2, space="PSUM") as yhps,
            tc.tile_pool(name="rbps", bufs=2, space="PSUM") as rbps,
        ):
            # ---- weights (f32r views of the fp32 DRAM data) ----
            wsqk = const.tile([128, NCB, 128], F32R, tag="wsqk", name="wsqk")
            nc.sync.dma_start(wsqk[:], wsqk_d[:, :].bitcast(F32R).rearrange("(cb p) d -> p cb d", p=128))
            wql = const.tile([128, NCB, 256], F32R, tag="wql", name="wql")
            nc.sync.dma_start(wql[:], wql_d[:, :].bitcast(F32R).rearrange("(cb p) d -> p cb d", p=128))
            wkl = const.tile([128, NCB, 256], F32R, tag="wkl", name="wkl")
            nc.sync.dma_start(wkl[:], wkl_d[:, :].bitcast(F32R).rearrange("(cb p) d -> p cb d", p=128))
            wv = const.tile([128, NCB, 256], F32R, tag="wv", name="wv")
            nc.sync.dma_start(wv[:], wv_d[:, :].bitcast(F32R).rearrange("(cb p) d -> p cb d", p=128))

            # ---- projection outputs (persist across both stages) ----
            # short heads: qts/kts [64, T], rows 0-31 head0, 32-63 head1
            # (separate tiles so score matmul lhsT/rhs base partitions align)
            qts = qkp.tile([64, T], F32R, tag="qts", name="qts")
            kts = qkp.tile([64, T], F32R, tag="kts", name="kts")
            qtl = [qkp.tile([128, T], F32R, tag=f"qtl{h}", name=f"qtl{h}") for h in range(2)]
            ktl = [qkp.tile([128, T], F32R, tag=f"ktl{h}", name=f"ktl{h}") for h in range(2)]
            # v tiles per head, [128, NT*VW]; col 64 of each block = 1.0
            vt = [vp.tile([128, NT * VW], F32R, tag=f"vt{i}", name=f"vt{i}") for i in range(4)]

            # ================= stage A: projections =================
            with tc.tile_pool(name="xtp", bufs=1) as xtp:
                xt = [xtp.tile([128, T], F32R, tag=f"xt{cb}", name=f"xt{cb}")
                      for cb in range(NCB)]
                # chunked loads so chunk-0 compute starts after 2MB, not 8MB
                for tch in range(T // 512):
                    for cb in range(NCB):
                        csl = (slice(None), slice(tch * 512, (tch + 1) * 512))
                        nc.sync.dma_start(
                            xt[cb][csl],
                            xt_d[cb * 128:(cb + 1) * 128, tch * 512:(tch + 1) * 512].bitcast(F32R))

                proj_jobs = [(wsqk, None, None)]
                for h in range(2):
                    proj_jobs.append((wql, h, qtl[h]))
                    proj_jobs.append((wkl, h, ktl[h]))
                for tch in range(T // 512):
                    for w, h, dst in proj_jobs:
                        ps = bigps.tile([128, 1024], F32, tag="bigps", name="bigps")
                        for cb in range(NCB):
                            lhsT = w[:, cb, :] if h is None else w[:, cb, h * 128:(h + 1) * 128]
                            nc.tensor.matmul(
                                ps[:, 0:512], lhsT, xt[cb][:, tch * 512:(tch + 1) * 512],
                                start=(cb == 0), stop=(cb == NCB - 1),
                            )
                        sl = (slice(None), slice(tch * 512, (tch + 1) * 512))
                        # scalar engine is idle during the projection phase
                        if dst is None:
                            nc.scalar.copy(qts[sl], ps[0:64, 0:512])
                            nc.scalar.copy(kts[sl], ps[64:128, 0:512])
                        else:
                            nc.scalar.copy(dst[sl], ps[:, 0:512])
                    for tb in range(4 * tch, 4 * tch + 4):
                        ps = bigps.tile([128, 1024], F32, tag="bigps", name="bigps")
                        for cb in range(NCB):
                            nc.tensor.matmul(
                                ps[:, 0:256], xt[cb][:, tb * 128:(tb + 1) * 128], wv[:, cb, :],
                                start=(cb == 0), stop=(cb == NCB - 1),
                            )
                        for i in range(4):
                            nc.scalar.copy(
                                vt[i][:, tb * VW: tb * VW + HD], ps[:, i * 64:(i + 1) * 64]
                            )

            # ============ stage B: attention + output projection ============
            with (
                tc.tile_pool(name="attnc", bufs=1) as attnc,
                tc.tile_pool(name="ptp", bufs=4) as ptp,
                tc.tile_pool(name="ytp", bufs=2) as ytp,
                tc.tile_pool(name="obp", bufs=3) as obp,
                tc.tile_pool(name="smallp", bufs=2) as smallp,
            ):
                wp0 = attnc.tile([128, C], F32R, tag="wp0", name="wp0")
                nc.sync.dma_start(wp0[:], wp_d[0:128, :].bitcast(F32R))
                wp1 = attnc.tile([128, C], F32R, tag="wp1", name="wp1")
                nc.sync.dma_start(wp1[:], wp_d[128:256, :].bitcast(F32R))
                band_s = attnc.tile([128, WIN_S + 896], F32R, tag="band_s", name="band_s")
                nc.sync.dma_start(band_s[:], bs_d[:, :].bitcast(F32R))
                band_l = attnc.tile([128, WIN_L + 896], F32R, tag="band_l", name="band_l")
                nc.sync.dma_start(band_l[:], bl_d[:, :].bitcast(F32R))
                onesr = attnc.tile([128, 64], F32, tag="onesr", name="onesr")
                nc.sync.dma_start(onesr[:], ones_d[:, :])
                # ones column of each v block (strided view [:, 64::65])
                for i in range(4):
                    v3 = vt[i][:, :].rearrange("p (nt vw) -> p nt vw", vw=VW)
                    nc.sync.dma_start(v3[:, :, HD], ones_d[:, 0:NT].bitcast(F32R))

                for qg in range(NG):
                    q0 = qg * 512
                    yts = [ytp.tile([128, 512], F32R, tag=f"yts{i}", name=f"yts{i}")
                           for i in range(2)]

                    heads = []
                    for h in range(2):  # short heads
                        heads.append((
                            lambda kb, h=h: kts[32 * h: 32 * h + 32, kb * 128:(kb + 1) * 128],
                            qts[32 * h: 32 * h + 32, q0: q0 + 512],
                            vt[h], WIN_S, scale_s, band_s, yts[0], 64 * h,
                        ))
                    for h in range(2):  # long heads
                        heads.append((
                            lambda kb, h=h: ktl[h][:, kb * 128:(kb + 1) * 128],
                            qtl[h][:, q0: q0 + 512],
                            vt[2 + h], WIN_L, scale_l, band_l, yts[1], 64 * h,
                        ))

                    s4 = smallp.tile([97, 512], F32, tag="s4", name="s4")
                    r4 = smallp.tile([97, 512], F32, tag="r4", name="r4")
                    yhs = []
                    for hi, (kt_ap, qt_ap, v_tile, win, scale, band, dest, poff) in enumerate(heads):
                        kb_lo = max(0, q0 - win) // 128
                        kb_hi = (q0 + 384) // 128
                        kbs = list(range(kb_lo, kb_hi + 1))
                        yh = yhps.tile([VW, 512], F32, tag="yh", name="yh")
                        # process key blocks in pairs sharing a 2-bank psum
                        # tile and a single exp instruction
                        pt_slices = []
                        for j in range(0, len(kbs), 2):
                            pair = kbs[j: j + 2]
                            wdt = 512 * len(pair)
                            st = bigps.tile([128, 1024], F32, tag="bigps", name="bigps")
                            for jj, kb in enumerate(pair):
                                nc.tensor.matmul(
                                    st[:, jj * 512:(jj + 1) * 512], kt_ap(kb), qt_ap,
                                    start=True, stop=True)
                            pt = ptp.tile([128, 1024], F32R, tag="pt", name="pt")
                            nc.scalar.activation(
                                pt[:, 0:wdt], st[:, 0:wdt],
                                mybir.ActivationFunctionType.Exp, scale=scale)
                            for jj, kb in enumerate(pair):
                                delta = kb * 128 - q0
                                psl = (slice(None), slice(jj * 512, (jj + 1) * 512))
                                if not (512 - win <= delta <= -128):
                                    off = 384 - delta
                                    eng = nc.vector if (kb + qg) % 2 == 0 else nc.gpsimd
                                    eng.tensor_tensor(out=pt[psl], in0=pt[psl],
                                                      in1=band[:, off: off + 512],
                                                      op=mybir.AluOpType.mult)
                                pt_slices.append((kb, pt, psl))
                        for i, (kb, pt, psl) in enumerate(pt_slices):
                            nc.tensor.matmul(
                                yh[:], v_tile[:, kb * VW:(kb + 1) * VW], pt[psl],
                                start=(i == 0), stop=(i == len(pt_slices) - 1),
                            )
                        nc.vector.tensor_copy(s4[32 * hi: 32 * hi + 1, :], yh[HD: HD + 1, :])
                        yv = smallp.tile([64, 512], F32, tag="yv", name="yv", bufs=4)
                        nc.vector.tensor_copy(yv[:], yh[0:HD, :])
                        yhs.append((yv, dest, poff))
                    nc.vector.reciprocal(r4[:], s4[:])
                    # matmul operand base partitions are limited to {0, 32, 64}:
                    # relocate head 3's reciprocal row to partition 0
                    r3 = smallp.tile([1, 512], F32, tag="r3", name="r3")
                    nc.vector.tensor_copy(r3[:], r4[96:97, :])
                    for hi, (yv, dest, poff) in enumerate(yhs):
                        rb = rbps.tile([64, 512], F32, tag="rb", name="rb")
                        rsrc = r3[0:1, :] if hi == 3 else r4[32 * hi: 32 * hi + 1, :]
                        osrc = onesr[0:1, :] if hi == 3 else onesr[32 * hi: 32 * hi + 1, :]
                        nc.tensor.matmul(rb[:], osrc, rsrc, start=True, stop=True)
                        with nc.allow_low_precision(reason="f32r rounding of attn out"):
                            nc.vector.tensor_mul(dest[poff: poff + 64, :], yv[:], rb[:])

                    for sub in range(4):
                        qs = q0 + sub * 128
                        ssl = (slice(None), slice(sub * 128, (sub + 1) * 128))
                        for nh in range(2):
                            po = bigps.tile([128, 1024], F32, tag="bigps", name="bigps")
                            nc.tensor.matmul(po[:, 0:512], yts[0][ssl], wp0[:, nh * 512:(nh + 1) * 512],
                                             start=True, stop=False)
                            nc.tensor.matmul(po[:, 0:512], yts[1][ssl], wp1[:, nh * 512:(nh + 1) * 512],
                                             start=False, stop=True)
                            ob = obp.tile([128, 512], F32, tag="ob", name="ob")
                            nc.vector.tensor_copy(ob[:], po[:, 0:512])
                            nc.sync.dma_start(out_d[qs: qs + 128, nh * 512:(nh + 1) * 512], ob[:])

    return nc


_PROGRAM = None


def _get_program() -> bass.Bass:
    global _PROGRAM
    if _PROGRAM is None:
        _PROGRAM = _build_program()
        _split_waits(_PROGRAM)
    return _PROGRAM


def _band_image(win: int) -> np.ndarray:
    """[128, win+896] 0/1 image: B[r, u] = 1 iff (u - 384 - r) in [0, win)."""
    u = np.arange(win + 896)[None, :]
    r = np.arange(128)[:, None]
    d = u - 384 - r
    return ((d >= 0) & (d < win)).astype(np.float32)


def make_in_maps(x, Wqk_short, Wv_short, Wqk_long, Wv_long, Wproj):
    """Host-side sharding: per-core input dict for core c = 4*b + g."""
    x = np.ascontiguousarray(np.asarray(x, dtype=np.float32))
    Wqk_short = np.asarray(Wqk_short, dtype=np.float32)
    Wv_short = np.asarray(Wv_short, dtype=np.float32)
    Wqk_long = np.asarray(Wqk_long, dtype=np.float32)
    Wv_long = np.asarray(Wv_long, dtype=np.float32)
    Wproj = np.asarray(Wproj, dtype=np.float32)
    assert x.shape == (B, T, C)

    xts = [np.ascontiguousarray(x[b].T) for b in range(B)]
    band_s = _band_image(WIN_S)
    band_l = _band_image(WIN_L)
    ones = np.ones((128, 64), dtype=np.float32)
    in_maps = []
    for c in range(N_CORES):
        b, g = divmod(c, 4)
        wsqk = np.ascontiguousarray(np.concatenate(
            [Wqk_short[:, g * 64:(g + 1) * 64],
             Wqk_short[:, 256 + g * 64: 256 + (g + 1) * 64]], axis=1))
        wql = np.ascontiguousarray(Wqk_long[:, g * 256:(g + 1) * 256])
        wkl = np.ascontiguousarray(Wqk_long[:, 1024 + g * 256: 1024 + (g + 1) * 256])
        wv = np.ascontiguousarray(np.concatenate(
            [Wv_short[:, g * 128:(g + 1) * 128],
             Wv_long[:, g * 128:(g + 1) * 128]], axis=1))
        wp = np.ascontiguousarray(np.concatenate(
            [Wproj[g * 128:(g + 1) * 128, :],
             Wproj[512 + g * 128: 512 + (g + 1) * 128, :]], axis=0))
        in_maps.append({
            "xt": xts[b], "wsqk": wsqk, "wql": wql, "wkl": wkl, "wv": wv, "wp": wp,
            "band_s": band_s, "band_l": band_l, "ones": ones,
        })
    return in_maps


def gather(results) -> np.ndarray:
    out = np.empty((B, T, C), dtype=np.float32)
    for b in range(B):
        acc = np.zeros((T, C), dtype=np.float64)
        for g in range(4):
            acc += results[4 * b + g]["out"]
        out[b] = acc.astype(np.float32)
    return out


def kernel(x, Wqk_short, Wv_short, Wqk_long, Wv_long, Wproj, **run_kwargs):
    nc = _get_program()
    in_maps = make_in_maps(x, Wqk_short, Wv_short, Wqk_long, Wv_long, Wproj)
    res = run_bass_kernel_spmd(nc, in_maps, core_ids=list(range(N_CORES)), **run_kwargs)
    out = gather(res.results)
    if run_kwargs:
        kernel.last_results = res
    return out



# revision 19
# speedup vs baseline: 1.5607x; 1.5607x over previous
"""Trainium2 Bass kernel for a two-window sparse causal self-attention block.

Model (B=2, T=2048, C=1024):
  - 8 "short" heads: d_qk=32,  window 256
  - 8 "long"  heads: d_qk=128, window 1024
  - value/output head dim 64, output projection C x C.

Sharding (8 cores): data-parallel over batch (2) x head-parallel over head
groups (4). Core c = 4*b + g handles batch b and heads {2g, 2g+1} of both the
short and long sets. Each core computes its 4 heads' attention plus the
corresponding 256 rows of Wproj, producing a partial [T, C] output (bf16); the
host sums the 4 partials per batch element in fp32.

v2 design notes (vs the first working version):
  - the tensor engine is kept continuously busy: per query group the
    (scores -> exp -> mask -> AV) chains of all heads are software-pipelined
    with an AV lag of D units, so the PE never sits in a cross-engine wait
    and its DVFS clock can ramp to 2.4 GHz.
  - long heads first, shorts after, so psum yh banks ring-recycle cleanly.
  - short heads use 256-wide query sub-blocks (window 256): computed score
    area per head drops from T*768 to T*512; N=256 keeps f32r at full rate.
  - exp outputs (softmax weights) and v tiles are bf16: AV matmuls run at
    1 cyc/row, DVE mask multiplies run in 2x packed mode, out DMA halves.
  - band masks are applied with ONE tensor_tensor per score strip using
    host-precomputed static 0/1 images concatenated per kb-offset; fully
    valid long pairs skip masking entirely.
  - softmax normalization: per-head reciprocal_approx_fast on the sums row,
    rank-1 matmul broadcast, one psum*psum multiply into the yts tile.
  - the output projection of group g is emitted interleaved into group g+1's
    score stream so it never stalls the group boundary.
"""

import contextlib
import math
from collections import deque


def _nullctx():
    return contextlib.nullcontext()

import numpy as np

import concourse.bass as bass
import concourse.mybir as mybir
import concourse.tile as tile
from concourse.bass_utils import run_bass_kernel_spmd

F32 = mybir.dt.float32
F32R = mybir.dt.float32r
BF16 = mybir.dt.bfloat16

B, T, C = 2, 2048, 1024
HS, DS = 8, 32
HL, DL = 8, 128
HD = 64
WIN_S, WIN_L = 256, 1024
NT = T // 128    # 16 t-blocks
NCB = C // 128   # 8 c-blocks
NG = T // 512    # 4 query groups
VW = HD + 1      # v columns + ones column for softmax sums
N_CORES = 8
AV_LAG = 3       # units between a strip's score matmuls and its AV matmuls


def _split_waits(nc: bass.Bass) -> int:
    """Walrus in this env accepts at most 1 sync wait per instruction.
    Hoist extra waits onto same-engine InstNoOp instructions placed just
    before the owning instruction (same-engine program order preserves the
    blocking semantics)."""
    import bass_rust

    n_added = 0
    for f in nc.m.functions:
        for bb in f.blocks:
            insts = bb.instructions
            if not any(inst.sync_info and len(inst.sync_info.on_wait) > 1
                       for inst in insts):
                continue
            new = []
            for inst in insts:
                si = inst.sync_info
                waits = list(si.on_wait) if si else []
                if len(waits) > 1:
                    for i, w in enumerate(waits[:-1]):
                        nop = mybir.InstNoOp(
                            name=f"{inst.name}_hw{i}",
                            sync_info=bass_rust.SyncInfo(on_wait=[w], on_update=[]),
                            bass_nofuse=True,
                            engine=inst.engine,
                        )
                        new.append(nop)
                        n_added += 1
                    inst.sync_info = bass_rust.SyncInfo(
                        on_wait=waits[-1:], on_update=list(si.on_update))
                new.append(inst)
            bb.instructions = new
    return n_added


def _patch_tile_drain():
    """This walrus build rejects >1 sync wait on the TileContext tail drain
    ("Too many sync wait commands"). Re-emit the drain's waits as individual
    wait_ge instructions on the sync engine."""
    import bass_rust
    from concourse.tile import ScopedClock, TileContext

    def _drain_and_barrier(self, tick_clock, wait_clock):
        nc = self.nc
        drain_inst = nc.sync.drain()
        wait_clock.add_sem_waits(
            drain_inst.ins, ScopedClock({None: tick_clock.global_clock})
        )
        si = drain_inst.ins.sync_info
        waits = list(si.on_wait) if si is not None else []
        if len(waits) > 1:
            drain_inst.ins.sync_info = bass_rust.SyncInfo(on_wait=[], on_update=[])
            sems = {h.name: h for h in self.sems.allocated().values()}
            for w in waits:
                nc.sync.wait_ge(sems[w.ant_name], w.wait_value)
        nc.all_engine_barrier()
        popped = nc._tile_sem_poison_stack.pop()
        assert popped is self._sem_poison
        nc.clear_and_free_semaphores(list(self.sems.allocated().values()))
        nc.all_engine_barrier()

    TileContext._drain_and_barrier = _drain_and_barrier


_patch_tile_drain()

# long pair images present in band_l, keyed by pair index j = (delta_a+1024)/256
_LONG_JMAP = {0: 0, 1: 1, 4: 2, 5: 3}


def _build_program() -> bass.Bass:
    nc = bass.Bass()

    xt_d = nc.dram_tensor("xt", [C, T], F32, kind="ExternalInput")
    wsqk_d = nc.dram_tensor("wsqk", [C, 128], F32, kind="ExternalInput")
    wql_d = nc.dram_tensor("wql", [C, 256], F32, kind="ExternalInput")
    wkl_d = nc.dram_tensor("wkl", [C, 256], F32, kind="ExternalInput")
    wv_d = nc.dram_tensor("wv", [C, 256], F32, kind="ExternalInput")
    wp_d = nc.dram_tensor("wp", [256, C], F32, kind="ExternalInput")
    bs_d = nc.dram_tensor("band_s", [128, 1024], BF16, kind="ExternalInput")
    bl_d = nc.dram_tensor("band_l", [128, 4096], BF16, kind="ExternalInput")
    out_d = nc.dram_tensor("out", [T, C], BF16, kind="ExternalOutput")

    scale_s = 1.0 / math.sqrt(DS)
    scale_l = 1.0 / math.sqrt(DL)

    with tile.TileContext(nc) as tc:
        with (
            tc.tile_pool(name="const", bufs=1) as const,
            tc.tile_pool(name="qkp", bufs=1) as qkp,
            tc.tile_pool(name="vp", bufs=1) as vp,
            tc.tile_pool(name="stp", bufs=2, space="PSUM") as stp,
            tc.tile_pool(name="ptp", bufs=5) as ptp,
        ):
            # ---- stage A weights (f32r views of the fp32 DRAM data) ----
            wsqk = const.tile([128, NCB, 128], F32R, tag="wsqk", name="wsqk")
            nc.sync.dma_start(wsqk[:], wsqk_d[:, :].bitcast(F32R).rearrange("(cb p) d -> p cb d", p=128))
            wql = const.tile([128, NCB, 256], F32R, tag="wql", name="wql")
            nc.sync.dma_start(wql[:], wql_d[:, :].bitcast(F32R).rearrange("(cb p) d -> p cb d", p=128))
            wkl = const.tile([128, NCB, 256], F32R, tag="wkl", name="wkl")
            nc.sync.dma_start(wkl[:], wkl_d[:, :].bitcast(F32R).rearrange("(cb p) d -> p cb d", p=128))
            wv = const.tile([128, NCB, 256], F32R, tag="wv", name="wv")
            nc.sync.dma_start(wv[:], wv_d[:, :].bitcast(F32R).rearrange("(cb p) d -> p cb d", p=128))

            # ---- projection outputs (persist across both stages) ----
            qts = qkp.tile([64, T], F32R, tag="qts", name="qts")
            kts = qkp.tile([64, T], F32R, tag="kts", name="kts")
            qtl = [qkp.tile([128, T], F32R, tag=f"qtl{h}", name=f"qtl{h}") for h in range(2)]
            ktl = [qkp.tile([128, T], F32R, tag=f"ktl{h}", name=f"ktl{h}") for h in range(2)]
            # v for all 4 heads (s0, s1, l0, l1), bf16, ones col at index HD
            vt = vp.tile([128, 4, NT, VW], BF16, tag="vt", name="vt")
            for i in range(4):
                nc.vector.memset(vt[:, i, :, HD], 1.0)

            # ================= stage A: projections =================
            with (
                tc.tile_pool(name="xtp", bufs=1) as xtp,
                tc.tile_pool(name="vps", bufs=2, space="PSUM") as vps,
            ):
                xt = xtp.tile([128, NCB, T], F32R, tag="xt", name="xt")
                for tch in range(T // 512):
                    sl = slice(tch * 512, (tch + 1) * 512)
                    nc.sync.dma_start(
                        xt[:, :, sl],
                        xt_d[:, sl].bitcast(F32R).rearrange("(cb p) t -> p cb t", p=128))

                proj_jobs = [(wsqk, None, None)]
                for h in range(2):
                    proj_jobs.append((wql, h, qtl[h]))
                    proj_jobs.append((wkl, h, ktl[h]))
                cp_alt = [0]

                def _acopy(dst_ap, src_ap, low=False):
                    eng = nc.scalar if cp_alt[0] % 2 == 0 else nc.vector
                    cp_alt[0] += 1
                    ctx = (nc.allow_low_precision(reason="bf16 v tiles") if low
                           else _nullctx())
                    with ctx:
                        if eng is nc.scalar:
                            eng.copy(dst_ap, src_ap)
                        else:
                            eng.tensor_copy(dst_ap, src_ap)

                for tch in range(T // 512):
                    sl = slice(tch * 512, (tch + 1) * 512)
                    for w, h, dst in proj_jobs:
                        ps = stp.tile([128, 1024], F32, tag="st", name="st")
                        for cb in range(NCB):
                            lhsT = w[:, cb, :] if h is None else w[:, cb, h * 128:(h + 1) * 128]
                            nc.tensor.matmul(
                                ps[:, 0:512], lhsT, xt[:, cb, sl],
                                start=(cb == 0), stop=(cb == NCB - 1),
                            )
                        if dst is None:
                            _acopy(qts[:, sl], ps[0:64, 0:512])
                            _acopy(kts[:, sl], ps[64:128, 0:512])
                        else:
                            _acopy(dst[:, sl], ps[:, 0:512])
                    for tb in range(4 * tch, 4 * tch + 4):
                        pv = vps.tile([128, 512], F32, tag="pv", name="pv")
                        for cb in range(NCB):
                            nc.tensor.matmul(
                                pv[:, 0:256], xt[:, cb, tb * 128:(tb + 1) * 128], wv[:, cb, :],
                                start=(cb == 0), stop=(cb == NCB - 1),
                            )
                        _acopy(vt[:, :, tb, 0:HD],
                               pv[:, 0:256].rearrange("p (i d) -> p i d", i=4),
                               low=True)

            # ============ stage B: attention + output projection ============
            with (
                tc.tile_pool(name="attnc", bufs=1) as attnc,
                tc.tile_pool(name="ytp", bufs=2) as ytp,
                tc.tile_pool(name="obp", bufs=4) as obp,
                tc.tile_pool(name="smallp", bufs=4) as smallp,
                tc.tile_pool(name="yhp", bufs=2, space="PSUM") as yhp,
                tc.tile_pool(name="onebank", bufs=2, space="PSUM") as onebank,
            ):
                wp0 = attnc.tile([128, C], F32R, tag="wp0", name="wp0")
                nc.sync.dma_start(wp0[:], wp_d[0:128, :].bitcast(F32R))
                wp1 = attnc.tile([128, C], F32R, tag="wp1", name="wp1")
                nc.sync.dma_start(wp1[:], wp_d[128:256, :].bitcast(F32R))
                band_s = attnc.tile([128, 1024], BF16, tag="band_s", name="band_s")
                nc.sync.dma_start(band_s[:], bs_d[:, :])
                band_l = attnc.tile([128, 4, 1024], BF16, tag="band_l", name="band_l")
                nc.sync.dma_start(band_l[:], bl_d[:, :].rearrange("p (j u) -> p j u", j=4))
                ones16 = attnc.tile([128, 64], BF16, tag="ones16", name="ones16")
                nc.vector.memset(ones16[:], 1.0)

                pend_wproj = []   # deferred output-projection emitters
                ob_alt = [0]      # rotates ob copies across scalar/vector
                msk_alt = [0]     # rotates mask multiplies across vector/gpsimd
                yv_alt = [0]      # rotates yv copies across scalar/vector

                def emit_wproj(yts_pair, q0):
                    ems = []
                    for sub in range(4):
                        for nh in range(2):
                            def em(sub=sub, nh=nh):
                                po = onebank.tile([128, 512], F32, tag="ob1", name="ob1")
                                ssl = (slice(None), slice(sub * 128, (sub + 1) * 128))
                                nc.tensor.matmul(po[:], yts_pair[0][ssl],
                                                 wp0[:, nh * 512:(nh + 1) * 512],
                                                 start=True, stop=False)
                                nc.tensor.matmul(po[:], yts_pair[1][ssl],
                                                 wp1[:, nh * 512:(nh + 1) * 512],
                                                 start=False, stop=True)
                                ob = obp.tile([128, 512], BF16, tag="ob", name="ob")
                                eng = nc.scalar if ob_alt[0] % 2 == 0 else nc.vector
                                ob_alt[0] += 1
                                with nc.allow_low_precision(reason="bf16 out"):
                                    if eng is nc.scalar:
                                        eng.copy(ob[:], po[:])
                                    else:
                                        eng.tensor_copy(ob[:], po[:])
                                qs = q0 + sub * 128
                                nc.sync.dma_start(
                                    out_d[qs:qs + 128, nh * 512:(nh + 1) * 512], ob[:])
                            ems.append(em)
                    return ems

                for qg in range(NG):
                    q0 = qg * 512
                    yts = [ytp.tile([128, 512], F32R, tag=f"yts{i}", name=f"yts{i}")
                           for i in range(2)]
                    # per-head state: [yh tile, avs_emitted, avs_total]
                    hstate = {}

                    norm = {"s4": None, "recs": []}
                    HIDX = {("L", 0): 0, ("L", 1): 1, ("S", 0): 2, ("S", 1): 3}

                    def phase1(key, yh, dest, poff):
                        # extract sums row + values, freeing the yh psum bank
                        i = HIDX[key]
                        if norm["s4"] is None:
                            norm["s4"] = smallp.tile([97, 512], F32, tag="s4",
                                                     name="s4")
                        s4 = norm["s4"]
                        nc.vector.tensor_copy(s4[32 * i:32 * i + 1, :],
                                              yh[HD:HD + 1, :])
                        yv = smallp.tile([64, 512], F32, tag="yv", name="yv")
                        eng = nc.scalar if yv_alt[0] % 2 == 0 else nc.vector
                        yv_alt[0] += 1
                        if eng is nc.scalar:
                            eng.copy(yv[:], yh[0:HD, :])
                        else:
                            eng.tensor_copy(yv[:], yh[0:HD, :])
                        norm["recs"].append((i, yv, dest, poff))

                    def phase2():
                        s4 = norm["s4"]
                        r4 = smallp.tile([97, 512], F32, tag="r4", name="r4")
                        nc.vector.reciprocal(r4[:], s4[:])
                        r16 = smallp.tile([97, 512], BF16, tag="r16", name="r16")
                        with nc.allow_low_precision(reason="bf16 recip"):
                            nc.vector.tensor_copy(r16[:], r4[:])
                        # matmul operand base partitions are limited to {0,32,64}:
                        # relocate head 3's reciprocal row to partition 0
                        r3 = smallp.tile([1, 512], BF16, tag="r3", name="r3")
                        nc.vector.tensor_copy(r3[:], r16[96:97, :])
                        for i, yv, dest, poff in norm["recs"]:
                            rb = onebank.tile([128, 512], F32, tag="ob1", name="ob1")
                            rsrc = r3[0:1, :] if i == 3 else r16[32 * i:32 * i + 1, :]
                            osrc = ones16[0:1, 0:64] if i == 3 else ones16[32 * i:32 * i + 1, 0:64]
                            nc.tensor.matmul(rb[0:64, :], osrc, rsrc,
                                             start=True, stop=True)
                            with nc.allow_low_precision(reason="f32r attn out"):
                                nc.vector.tensor_mul(dest[poff:poff + 64, :],
                                                     yv[:], rb[0:64, :])

                    units = []
                    # ---- long heads, h0/h1 interleaved per kb-pair ----
                    kb_lo = max(0, (q0 - WIN_L) // 128)
                    kb_hi = (q0 + 384) // 128
                    kbs_l = list(range(kb_lo, kb_hi + 1))
                    pairs = [(kbs_l[j], kbs_l[j + 1]) for j in range(0, len(kbs_l), 2)]
                    for pi, pair in enumerate(pairs):
                        for h in range(2):
                            units.append(("L", h, pair, pi == 0, pi == len(pairs) - 1))
                    # ---- short heads, 256-wide sub-blocks ----
                    sq_kbs = []
                    for sq in range(2):
                        q0s = q0 + 256 * sq
                        lo = max(0, (q0s - WIN_S) // 128)
                        hi = (q0s + 128) // 128
                        sq_kbs.append(list(range(lo, hi + 1)))
                    for sq in range(2):
                        for h in range(2):
                            units.append(("S", h, sq, sq == 0, sq == 1))

                    def emit_scores(u):
                        kind = u[0]
                        if kind == "L":
                            _, h, pair, _, _ = u
                            st = stp.tile([128, 1024], F32, tag="st", name="st")
                            for jj, kb in enumerate(pair):
                                nc.tensor.matmul(
                                    st[:, jj * 512:(jj + 1) * 512],
                                    ktl[h][:, kb * 128:(kb + 1) * 128],
                                    qtl[h][:, q0:q0 + 512], start=True, stop=True)
                            pt = ptp.tile([128, 1024], BF16, tag="pt", name="pt")
                            with nc.allow_low_precision(reason="bf16 softmax wts"):
                                nc.scalar.activation(
                                    pt[:], st[:],
                                    mybir.ActivationFunctionType.Exp, scale=scale_l)
                            j = (pair[0] * 128 - q0 + 1024) // 256
                            if j in _LONG_JMAP:
                                eng = nc.vector if msk_alt[0] % 2 == 0 else nc.gpsimd
                                msk_alt[0] += 1
                                eng.tensor_tensor(
                                    out=pt[:], in0=pt[:],
                                    in1=band_l[:, _LONG_JMAP[j], :],
                                    op=mybir.AluOpType.mult)
                            return pt
                        else:
                            _, h, sq, _, _ = u
                            q0s = q0 + 256 * sq
                            kbs = sq_kbs[sq]
                            wdt = 256 * len(kbs)
                            st = stp.tile([128, 1024], F32, tag="st", name="st")
                            for jj, kb in enumerate(kbs):
                                nc.tensor.matmul(
                                    st[:, jj * 256:(jj + 1) * 256],
                                    kts[32 * h:32 * h + 32, kb * 128:(kb + 1) * 128],
                                    qts[32 * h:32 * h + 32, q0s:q0s + 256],
                                    start=True, stop=True)
                            pt = ptp.tile([128, 1024], BF16, tag="pt", name="pt")
                            with nc.allow_low_precision(reason="bf16 softmax wts"):
                                nc.scalar.activation(
                                    pt[:, 0:wdt], st[:, 0:wdt],
                                    mybir.ActivationFunctionType.Exp, scale=scale_s)
                            eng = nc.vector if msk_alt[0] % 2 == 0 else nc.gpsimd
                            msk_alt[0] += 1
                            eng.tensor_tensor(
                                out=pt[:, 0:wdt], in0=pt[:, 0:wdt],
                                in1=band_s[:, 1024 - wdt:1024],
                                op=mybir.AluOpType.mult)
                            return pt

                    def emit_av(u, pt):
                        kind = u[0]
                        if kind == "L":
                            _, h, pair, first, last = u
                            key = ("L", h)
                            if key not in hstate:
                                hstate[key] = yhp.tile([VW, 512], F32, tag="yh",
                                                       name="yh")
                            yh = hstate[key]
                            for jj, kb in enumerate(pair):
                                nc.tensor.matmul(
                                    yh[:], vt[:, 2 + h, kb, :],
                                    pt[:, jj * 512:(jj + 1) * 512],
                                    start=(first and jj == 0),
                                    stop=(last and jj == len(pair) - 1))
                            if last:
                                phase1(("L", h), yh, yts[1], 64 * h)
                        else:
                            _, h, sq, first, last = u
                            key = ("S", h)
                            if key not in hstate:
                                hstate[key] = yhp.tile([VW, 512], F32, tag="yh",
                                                       name="yh")
                            yh = hstate[key]
                            kbs = sq_kbs[sq]
                            for jj, kb in enumerate(kbs):
                                nc.tensor.matmul(
                                    yh[:, sq * 256:(sq + 1) * 256],
                                    vt[:, h, kb, :],
                                    pt[:, jj * 256:(jj + 1) * 256],
                                    start=(first and jj == 0),
                                    stop=(last and jj == len(kbs) - 1))
                            if last:
                                phase1(("S", h), yh, yts[0], 64 * h)

                    pend_av = deque()
                    for u in units:
                        pt = emit_scores(u)
                        pend_av.append((u, pt))
                        if pend_wproj:
                            pend_wproj.pop(0)()
                        if len(pend_av) > AV_LAG:
                            emit_av(*pend_av.popleft())
                    while pend_av:
                        emit_av(*pend_av.popleft())
                    phase2()
                    while pend_wproj:
                        pend_wproj.pop(0)()
                    pend_wproj = emit_wproj(yts, q0)
                while pend_wproj:
                    pend_wproj.pop(0)()

    return nc


_PROGRAM = None


def _get_program() -> bass.Bass:
    global _PROGRAM
    if _PROGRAM is None:
        _PROGRAM = _build_program()
        _split_waits(_PROGRAM)
    return _PROGRAM


def _pattern(delta: int, qw: int, win: int) -> np.ndarray:
    """[128, qw] 0/1 validity image for a key block at offset delta from the
    query block: cell (p, c) valid iff 0 <= (c - delta - p) < win."""
    p = np.arange(128)[:, None]
    c = np.arange(qw)[None, :]
    d = c - delta - p
    return ((d >= 0) & (d < win)).astype(np.float32)


def _band_images():
    import ml_dtypes
    bs = np.concatenate([_pattern(d, 256, WIN_S) for d in (-256, -128, 0, 128)],
                        axis=1)
    bl = np.concatenate(
        [np.concatenate([_pattern(da, 512, WIN_L), _pattern(da + 128, 512, WIN_L)],
                        axis=1)
         for da in (-1024, -768, 0, 256)], axis=1)
    return (np.ascontiguousarray(bs.astype(ml_dtypes.bfloat16)),
            np.ascontiguousarray(bl.astype(ml_dtypes.bfloat16)))


def make_in_maps(x, Wqk_short, Wv_short, Wqk_long, Wv_long, Wproj):
    """Host-side sharding: per-core input dict for core c = 4*b + g."""
    x = np.ascontiguousarray(np.asarray(x, dtype=np.float32))
    Wqk_short = np.asarray(Wqk_short, dtype=np.float32)
    Wv_short = np.asarray(Wv_short, dtype=np.float32)
    Wqk_long = np.asarray(Wqk_long, dtype=np.float32)
    Wv_long = np.asarray(Wv_long, dtype=np.float32)
    Wproj = np.asarray(Wproj, dtype=np.float32)
    assert x.shape == (B, T, C)

    xts = [np.ascontiguousarray(x[b].T) for b in range(B)]
    band_s, band_l = _band_images()
    in_maps = []
    for c in range(N_CORES):
        b, g = divmod(c, 4)
        wsqk = np.ascontiguousarray(np.concatenate(
            [Wqk_short[:, g * 64:(g + 1) * 64],
             Wqk_short[:, 256 + g * 64: 256 + (g + 1) * 64]], axis=1))
        wql = np.ascontiguousarray(Wqk_long[:, g * 256:(g + 1) * 256])
        wkl = np.ascontiguousarray(Wqk_long[:, 1024 + g * 256: 1024 + (g + 1) * 256])
        wv = np.ascontiguousarray(np.concatenate(
            [Wv_short[:, g * 128:(g + 1) * 128],
             Wv_long[:, g * 128:(g + 1) * 128]], axis=1))
        wp = np.ascontiguousarray(np.concatenate(
            [Wproj[g * 128:(g + 1) * 128, :],
             Wproj[512 + g * 128: 512 + (g + 1) * 128, :]], axis=0))
        in_maps.append({
            "xt": xts[b], "wsqk": wsqk, "wql": wql, "wkl": wkl, "wv": wv, "wp": wp,
            "band_s": band_s, "band_l": band_l,
        })
    return in_maps


def gather(results) -> np.ndarray:
    out = np.empty((B, T, C), dtype=np.float32)
    for b in range(B):
        acc = np.zeros((T, C), dtype=np.float32)
        for g in range(4):
            acc += np.asarray(results[4 * b + g]["out"], dtype=np.float32)
        out[b] = acc
    return out


def kernel(x, Wqk_short, Wv_short, Wqk_long, Wv_long, Wproj, **run_kwargs):
    nc = _get_program()
    in_maps = make_in_maps(x, Wqk_short, Wv_short, Wqk_long, Wv_long, Wproj)
    res = run_bass_kernel_spmd(nc, in_maps, core_ids=list(range(N_CORES)), **run_kwargs)
    out = gather(res.results)
    if run_kwargs:
        kernel.last_results = res
    return out


# revision 24
# speedup vs baseline: 1.6510x; 1.0579x over previous
"""Trainium2 Bass kernel for a two-window sparse causal self-attention block.

Model (B=2, T=2048, C=1024):
  - 8 "short" heads: d_qk=32,  window 256
  - 8 "long"  heads: d_qk=128, window 1024
  - value/output head dim 64, output projection C x C.

Sharding (8 cores): data-parallel over batch (2) x head-parallel over head
groups (4). Core c = 4*b + g handles batch b and heads {2g, 2g+1} of both the
short and long sets. Each core computes its 4 heads' attention plus the
corresponding 256 rows of Wproj, producing a partial [T, C] output (bf16); the
host sums the 4 partials per batch element in fp32.

v2 design notes (vs the first working version):
  - the tensor engine is kept continuously busy: per query group the
    (scores -> exp -> mask -> AV) chains of all heads are software-pipelined
    with an AV lag of D units, so the PE never sits in a cross-engine wait
    and its DVFS clock can ramp to 2.4 GHz.
  - long heads first, shorts after, so psum yh banks ring-recycle cleanly.
  - short heads use 256-wide query sub-blocks (window 256): computed score
    area per head drops from T*768 to T*512; N=256 keeps f32r at full rate.
  - exp outputs (softmax weights) and v tiles are bf16: AV matmuls run at
    1 cyc/row, DVE mask multiplies run in 2x packed mode, out DMA halves.
  - band masks are applied with ONE tensor_tensor per score strip using
    host-precomputed static 0/1 images concatenated per kb-offset; fully
    valid long pairs skip masking entirely.
  - softmax normalization: per-head reciprocal_approx_fast on the sums row,
    rank-1 matmul broadcast, one psum*psum multiply into the yts tile.
  - the output projection of group g is emitted interleaved into group g+1's
    score stream so it never stalls the group boundary.
"""

import contextlib
import math
from collections import deque


def _nullctx():
    return contextlib.nullcontext()

import numpy as np

import concourse.bass as bass
import concourse.mybir as mybir
import concourse.tile as tile
from concourse.bass_utils import run_bass_kernel_spmd

F32 = mybir.dt.float32
F32R = mybir.dt.float32r
BF16 = mybir.dt.bfloat16

B, T, C = 2, 2048, 1024
HS, DS = 8, 32
HL, DL = 8, 128
HD = 64
WIN_S, WIN_L = 256, 1024
NT = T // 128    # 16 t-blocks
NCB = C // 128   # 8 c-blocks
NG = T // 512    # 4 query groups
VW = HD + 1      # v columns + ones column for softmax sums
N_CORES = 8
AV_LAG = 4       # units between a strip's score matmuls and its AV matmuls


def _split_waits(nc: bass.Bass) -> int:
    """Walrus in this env accepts at most 1 sync wait per instruction.
    Hoist extra waits onto same-engine InstNoOp instructions placed just
    before the owning instruction (same-engine program order preserves the
    blocking semantics)."""
    import bass_rust

    n_added = 0
    for f in nc.m.functions:
        for bb in f.blocks:
            insts = bb.instructions
            if not any(inst.sync_info and len(inst.sync_info.on_wait) > 1
                       for inst in insts):
                continue
            new = []
            for inst in insts:
                si = inst.sync_info
                waits = list(si.on_wait) if si else []
                if len(waits) > 1:
                    for i, w in enumerate(waits[:-1]):
                        nop = mybir.InstNoOp(
                            name=f"{inst.name}_hw{i}",
                            sync_info=bass_rust.SyncInfo(on_wait=[w], on_update=[]),
                            bass_nofuse=True,
                            engine=inst.engine,
                        )
                        new.append(nop)
                        n_added += 1
                    inst.sync_info = bass_rust.SyncInfo(
                        on_wait=waits[-1:], on_update=list(si.on_update))
                new.append(inst)
            bb.instructions = new
    return n_added


def _patch_tile_drain():
    """This walrus build rejects >1 sync wait on the TileContext tail drain
    ("Too many sync wait commands"). Re-emit the drain's waits as individual
    wait_ge instructions on the sync engine."""
    import bass_rust
    from concourse.tile import ScopedClock, TileContext

    def _drain_and_barrier(self, tick_clock, wait_clock):
        nc = self.nc
        drain_inst = nc.sync.drain()
        wait_clock.add_sem_waits(
            drain_inst.ins, ScopedClock({None: tick_clock.global_clock})
        )
        si = drain_inst.ins.sync_info
        waits = list(si.on_wait) if si is not None else []
        if len(waits) > 1:
            drain_inst.ins.sync_info = bass_rust.SyncInfo(on_wait=[], on_update=[])
            sems = {h.name: h for h in self.sems.allocated().values()}
            for w in waits:
                nc.sync.wait_ge(sems[w.ant_name], w.wait_value)
        nc.all_engine_barrier()
        popped = nc._tile_sem_poison_stack.pop()
        assert popped is self._sem_poison
        nc.clear_and_free_semaphores(list(self.sems.allocated().values()))
        nc.all_engine_barrier()

    TileContext._drain_and_barrier = _drain_and_barrier


_patch_tile_drain()

# long pair images present in band_l, keyed by pair index j = (delta_a+1024)/256
_LONG_JMAP = {0: 0, 1: 1, 4: 2, 5: 3}


def _build_program() -> bass.Bass:
    nc = bass.Bass()

    xt_d = nc.dram_tensor("xt", [C, T], F32, kind="ExternalInput")
    wsqk_d = nc.dram_tensor("wsqk", [C, 128], F32, kind="ExternalInput")
    wql_d = nc.dram_tensor("wql", [C, 256], F32, kind="ExternalInput")
    wkl_d = nc.dram_tensor("wkl", [C, 256], F32, kind="ExternalInput")
    wv_d = nc.dram_tensor("wv", [C, 256], F32, kind="ExternalInput")
    wp_d = nc.dram_tensor("wp", [256, C], F32, kind="ExternalInput")
    bs_d = nc.dram_tensor("band_s", [128, 1024], BF16, kind="ExternalInput")
    bl_d = nc.dram_tensor("band_l", [128, 4096], BF16, kind="ExternalInput")
    out_d = nc.dram_tensor("out", [T, C], BF16, kind="ExternalOutput")

    scale_s = 1.0 / math.sqrt(DS)
    scale_l = 1.0 / math.sqrt(DL)

    with tile.TileContext(nc) as tc:
        with (
            tc.tile_pool(name="const", bufs=1) as const,
            tc.tile_pool(name="qkp", bufs=1) as qkp,
            tc.tile_pool(name="vp", bufs=1) as vp,
            tc.tile_pool(name="stp", bufs=2, space="PSUM") as stp,
            tc.tile_pool(name="ptp", bufs=6) as ptp,
        ):
            # ---- stage A weights (f32r views of the fp32 DRAM data) ----
            # DMA order matters for startup: wsqk + xt chunk 0 first so the
            # first projection matmul can start ~8us in; the remaining
            # weights stream in while chunk-0 compute runs.
            wsqk = const.tile([128, NCB, 128], F32R, tag="wsqk", name="wsqk")
            nc.sync.dma_start(wsqk[:], wsqk_d[:, :].bitcast(F32R).rearrange("(cb p) d -> p cb d", p=128))
            wql = const.tile([128, NCB, 256], F32R, tag="wql", name="wql")
            wkl = const.tile([128, NCB, 256], F32R, tag="wkl", name="wkl")
            wv = const.tile([128, NCB, 256], F32R, tag="wv", name="wv")

            # ---- projection outputs (persist across both stages) ----
            qts = qkp.tile([64, T], F32R, tag="qts", name="qts")
            kts = qkp.tile([64, T], F32R, tag="kts", name="kts")
            qtl = [qkp.tile([128, T], F32R, tag=f"qtl{h}", name=f"qtl{h}") for h in range(2)]
            ktl = [qkp.tile([128, T], F32R, tag=f"ktl{h}", name=f"ktl{h}") for h in range(2)]
            # v for all 4 heads (s0, s1, l0, l1), bf16, ones col at index HD
            vt = vp.tile([128, 4, NT, VW], BF16, tag="vt", name="vt")
            for i in range(4):
                nc.vector.memset(vt[:, i, :, HD], 1.0)

            # ================= stage A: projections =================
            with (
                tc.tile_pool(name="xtp", bufs=1) as xtp,
                tc.tile_pool(name="vps", bufs=2, space="PSUM") as vps,
            ):
                xt = xtp.tile([128, NCB, T], F32R, tag="xt", name="xt")
                for tch in range(T // 512):
                    sl = slice(tch * 512, (tch + 1) * 512)
                    nc.sync.dma_start(
                        xt[:, :, sl],
                        xt_d[:, sl].bitcast(F32R).rearrange("(cb p) t -> p cb t", p=128))
                    if tch == 0:
                        nc.sync.dma_start(wql[:], wql_d[:, :].bitcast(F32R).rearrange("(cb p) d -> p cb d", p=128))
                        nc.sync.dma_start(wkl[:], wkl_d[:, :].bitcast(F32R).rearrange("(cb p) d -> p cb d", p=128))
                        nc.sync.dma_start(wv[:], wv_d[:, :].bitcast(F32R).rearrange("(cb p) d -> p cb d", p=128))

                proj_jobs = [(wsqk, None, None)]
                for h in range(2):
                    proj_jobs.append((wql, h, qtl[h]))
                    proj_jobs.append((wkl, h, ktl[h]))
                cp_alt = [0]

                def _acopy(dst_ap, src_ap, low=False):
                    eng = nc.scalar if cp_alt[0] % 2 == 0 else nc.vector
                    cp_alt[0] += 1
                    ctx = (nc.allow_low_precision(reason="bf16 v tiles") if low
                           else _nullctx())
                    with ctx:
                        if eng is nc.scalar:
                            eng.copy(dst_ap, src_ap)
                        else:
                            eng.tensor_copy(dst_ap, src_ap)

                for tch in range(T // 512):
                    sl = slice(tch * 512, (tch + 1) * 512)
                    for w, h, dst in proj_jobs:
                        ps = stp.tile([128, 1024], F32, tag="st", name="st")
                        for cb in range(NCB):
                            lhsT = w[:, cb, :] if h is None else w[:, cb, h * 128:(h + 1) * 128]
                            nc.tensor.matmul(
                                ps[:, 0:512], lhsT, xt[:, cb, sl],
                                start=(cb == 0), stop=(cb == NCB - 1),
                            )
                        if dst is None:
                            _acopy(qts[:, sl], ps[0:64, 0:512])
                            _acopy(kts[:, sl], ps[64:128, 0:512])
                        else:
                            _acopy(dst[:, sl], ps[:, 0:512])
                    for tb in range(4 * tch, 4 * tch + 4):
                        pv = vps.tile([128, 512], F32, tag="pv", name="pv")
                        for cb in range(NCB):
                            nc.tensor.matmul(
                                pv[:, 0:256], xt[:, cb, tb * 128:(tb + 1) * 128], wv[:, cb, :],
                                start=(cb == 0), stop=(cb == NCB - 1),
                            )
                        _acopy(vt[:, :, tb, 0:HD],
                               pv[:, 0:256].rearrange("p (i d) -> p i d", i=4),
                               low=True)

            # ============ stage B: attention + output projection ============
            with (
                tc.tile_pool(name="attnc", bufs=1) as attnc,
                tc.tile_pool(name="ytp", bufs=2) as ytp,
                tc.tile_pool(name="obp", bufs=4) as obp,
                tc.tile_pool(name="smallp", bufs=4) as smallp,
                tc.tile_pool(name="yhp", bufs=2, space="PSUM") as yhp,
                tc.tile_pool(name="onebank", bufs=2, space="PSUM") as onebank,
            ):
                wp0 = attnc.tile([128, C], F32R, tag="wp0", name="wp0")
                nc.sync.dma_start(wp0[:], wp_d[0:128, :].bitcast(F32R))
                wp1 = attnc.tile([128, C], F32R, tag="wp1", name="wp1")
                nc.sync.dma_start(wp1[:], wp_d[128:256, :].bitcast(F32R))
                band_s = attnc.tile([128, 1024], BF16, tag="band_s", name="band_s")
                nc.sync.dma_start(band_s[:], bs_d[:, :])
                band_l = attnc.tile([128, 4, 1024], BF16, tag="band_l", name="band_l")
                nc.sync.dma_start(band_l[:], bl_d[:, :].rearrange("p (j u) -> p j u", j=4))
                ones16 = attnc.tile([128, 64], BF16, tag="ones16", name="ones16")
                nc.vector.memset(ones16[:], 1.0)

                pend_wproj = []   # deferred output-projection emitters
                ob_alt = [0]      # rotates ob copies across scalar/vector
                msk_alt = [0]     # rotates mask multiplies across vector/gpsimd
                yv_alt = [0]      # rotates yv copies across scalar/vector

                def emit_wproj(yts_pair, q0):
                    ems = []
                    for sub in range(4):
                        for nh in range(2):
                            def em(sub=sub, nh=nh):
                                po = onebank.tile([128, 512], F32, tag="ob1", name="ob1")
                                ssl = (slice(None), slice(sub * 128, (sub + 1) * 128))
                                nc.tensor.matmul(po[:], yts_pair[0][ssl],
                                                 wp0[:, nh * 512:(nh + 1) * 512],
                                                 start=True, stop=False)
                                nc.tensor.matmul(po[:], yts_pair[1][ssl],
                                                 wp1[:, nh * 512:(nh + 1) * 512],
                                                 start=False, stop=True)
                                ob = obp.tile([128, 512], BF16, tag="ob", name="ob")
                                eng = nc.scalar if ob_alt[0] % 2 == 0 else nc.vector
                                ob_alt[0] += 1
                                with nc.allow_low_precision(reason="bf16 out"):
                                    if eng is nc.scalar:
                                        eng.copy(ob[:], po[:])
                                    else:
                                        eng.tensor_copy(ob[:], po[:])
                                qs = q0 + sub * 128
                                nc.sync.dma_start(
                                    out_d[qs:qs + 128, nh * 512:(nh + 1) * 512], ob[:])
                            ems.append(em)
                    return ems

                for qg in range(NG):
                    q0 = qg * 512
                    yts = [ytp.tile([128, 512], F32R, tag=f"yts{i}", name=f"yts{i}")
                           for i in range(2)]
                    # per-head state: [yh tile, avs_emitted, avs_total]
                    hstate = {}

                    norm = {"s4": None, "recs": []}
                    HIDX = {("L", 0): 0, ("L", 1): 1, ("S", 0): 2, ("S", 1): 3}

                    def phase1(key, yh, dest, poff):
                        # extract sums row + values, freeing the yh psum bank
                        i = HIDX[key]
                        if norm["s4"] is None:
                            norm["s4"] = smallp.tile([97, 512], F32, tag="s4",
                                                     name="s4")
                        s4 = norm["s4"]
                        nc.vector.tensor_copy(s4[32 * i:32 * i + 1, :],
                                              yh[HD:HD + 1, :])
                        yv = smallp.tile([64, 512], F32, tag="yv", name="yv")
                        eng = nc.scalar if yv_alt[0] % 2 == 0 else nc.vector
                        yv_alt[0] += 1
                        if eng is nc.scalar:
                            eng.copy(yv[:], yh[0:HD, :])
                        else:
                            eng.tensor_copy(yv[:], yh[0:HD, :])
                        norm["recs"].append((i, yv, dest, poff))

                    def phase2_emitters():
                        # deferred into the next group's unit stream so the
                        # recip chain never blocks the tensor engine
                        nrm = dict(norm)
                        state = {}

                        def em_recip():
                            s4 = nrm["s4"]
                            r4 = smallp.tile([97, 512], F32, tag="r4", name="r4")
                            nc.vector.reciprocal(r4[:], s4[:])
                            r16 = smallp.tile([97, 512], BF16, tag="r16", name="r16")
                            with nc.allow_low_precision(reason="bf16 recip"):
                                nc.vector.tensor_copy(r16[:], r4[:])
                            # matmul base partitions are limited to {0,32,64}:
                            # relocate head 3's reciprocal row to partition 0
                            r3 = smallp.tile([1, 512], BF16, tag="r3", name="r3")
                            nc.vector.tensor_copy(r3[:], r16[96:97, :])
                            state["r16"], state["r3"] = r16, r3

                        ems = [em_recip]
                        for rec in nrm["recs"]:
                            def em_norm(rec=rec):
                                i, yv, dest, poff = rec
                                r16, r3 = state["r16"], state["r3"]
                                rb = onebank.tile([128, 512], F32, tag="ob1",
                                                  name="ob1")
                                rsrc = r3[0:1, :] if i == 3 else r16[32 * i:32 * i + 1, :]
                                osrc = ones16[0:1, 0:64] if i == 3 else ones16[32 * i:32 * i + 1, 0:64]
                                nc.tensor.matmul(rb[0:64, :], osrc, rsrc,
                                                 start=True, stop=True)
                                with nc.allow_low_precision(reason="f32r attn out"):
                                    nc.vector.tensor_mul(dest[poff:poff + 64, :],
                                                         yv[:], rb[0:64, :])
                            ems.append(em_norm)
                        return ems

                    units = []
                    # ---- long heads, h0/h1 interleaved per kb-pair ----
                    kb_lo = max(0, (q0 - WIN_L) // 128)
                    kb_hi = (q0 + 384) // 128
                    kbs_l = list(range(kb_lo, kb_hi + 1))
                    pairs = [(kbs_l[j], kbs_l[j + 1]) for j in range(0, len(kbs_l), 2)]
                    for pi, pair in enumerate(pairs):
                        for h in range(2):
                            units.append(("L", h, pair, pi == 0, pi == len(pairs) - 1))
                    # ---- short heads, 256-wide sub-blocks ----
                    sq_kbs = []
                    for sq in range(2):
                        q0s = q0 + 256 * sq
                        lo = max(0, (q0s - WIN_S) // 128)
                        hi = (q0s + 128) // 128
                        sq_kbs.append(list(range(lo, hi + 1)))
                    for sq in range(2):
                        for h in range(2):
                            units.append(("S", h, sq, sq == 0, sq == 1))

                    def emit_scores(u):
                        kind = u[0]
                        if kind == "L":
                            _, h, pair, _, _ = u
                            st = stp.tile([128, 1024], F32, tag="st", name="st")
                            for jj, kb in enumerate(pair):
                                nc.tensor.matmul(
                                    st[:, jj * 512:(jj + 1) * 512],
                                    ktl[h][:, kb * 128:(kb + 1) * 128],
                                    qtl[h][:, q0:q0 + 512], start=True, stop=True)
                            pt = ptp.tile([128, 1024], BF16, tag="pt", name="pt")
                            with nc.allow_low_precision(reason="bf16 softmax wts"):
                                nc.scalar.activation(
                                    pt[:], st[:],
                                    mybir.ActivationFunctionType.Exp, scale=scale_l)
                            j = (pair[0] * 128 - q0 + 1024) // 256
                            if j in _LONG_JMAP:
                                eng = nc.vector if msk_alt[0] % 2 == 0 else nc.gpsimd
                                msk_alt[0] += 1
                                eng.tensor_tensor(
                                    out=pt[:], in0=pt[:],
                                    in1=band_l[:, _LONG_JMAP[j], :],
                                    op=mybir.AluOpType.mult)
                            return pt
                        else:
                            _, h, sq, _, _ = u
                            q0s = q0 + 256 * sq
                            kbs = sq_kbs[sq]
                            wdt = 256 * len(kbs)
                            st = stp.tile([128, 1024], F32, tag="st", name="st")
                            for jj, kb in enumerate(kbs):
                                nc.tensor.matmul(
                                    st[:, jj * 256:(jj + 1) * 256],
                                    kts[32 * h:32 * h + 32, kb * 128:(kb + 1) * 128],
                                    qts[32 * h:32 * h + 32, q0s:q0s + 256],
                                    start=True, stop=True)
                            pt = ptp.tile([128, 1024], BF16, tag="pt", name="pt")
                            with nc.allow_low_precision(reason="bf16 softmax wts"):
                                nc.scalar.activation(
                                    pt[:, 0:wdt], st[:, 0:wdt],
                                    mybir.ActivationFunctionType.Exp, scale=scale_s)
                            eng = nc.vector if msk_alt[0] % 2 == 0 else nc.gpsimd
                            msk_alt[0] += 1
                            eng.tensor_tensor(
                                out=pt[:, 0:wdt], in0=pt[:, 0:wdt],
                                in1=band_s[:, 1024 - wdt:1024],
                                op=mybir.AluOpType.mult)
                            return pt

                    def emit_av(u, pt):
                        kind = u[0]
                        if kind == "L":
                            _, h, pair, first, last = u
                            key = ("L", h)
                            if key not in hstate:
                                hstate[key] = yhp.tile([VW, 512], F32, tag="yh",
                                                       name="yh")
                            yh = hstate[key]
                            for jj, kb in enumerate(pair):
                                nc.tensor.matmul(
                                    yh[:], vt[:, 2 + h, kb, :],
                                    pt[:, jj * 512:(jj + 1) * 512],
                                    start=(first and jj == 0),
                                    stop=(last and jj == len(pair) - 1))
                            if last:
                                phase1(("L", h), yh, yts[1], 64 * h)
                        else:
                            _, h, sq, first, last = u
                            key = ("S", h)
                            if key not in hstate:
                                hstate[key] = yhp.tile([VW, 512], F32, tag="yh",
                                                       name="yh")
                            yh = hstate[key]
                            kbs = sq_kbs[sq]
                            for jj, kb in enumerate(kbs):
                                nc.tensor.matmul(
                                    yh[:, sq * 256:(sq + 1) * 256],
                                    vt[:, h, kb, :],
                                    pt[:, jj * 256:(jj + 1) * 256],
                                    start=(first and jj == 0),
                                    stop=(last and jj == len(kbs) - 1))
                            if last:
                                phase1(("S", h), yh, yts[0], 64 * h)

                    pend_av = deque()
                    for u in units:
                        pt = emit_scores(u)
                        pend_av.append((u, pt))
                        if pend_wproj:
                            pend_wproj.pop(0)()
                        if len(pend_av) > AV_LAG:
                            emit_av(*pend_av.popleft())
                    while pend_av:
                        emit_av(*pend_av.popleft())
                    while pend_wproj:
                        pend_wproj.pop(0)()
                    pend_wproj = phase2_emitters() + emit_wproj(yts, q0)
                while pend_wproj:
                    pend_wproj.pop(0)()

    return nc


_PROGRAM = None


def _get_program() -> bass.Bass:
    global _PROGRAM
    if _PROGRAM is None:
        _PROGRAM = _build_program()
        _split_waits(_PROGRAM)
    return _PROGRAM


def _pattern(delta: int, qw: int, win: int) -> np.ndarray:
    """[128, qw] 0/1 validity image for a key block at offset delta from the
    query block: cell (p, c) valid iff 0 <= (c - delta - p) < win."""
    p = np.arange(128)[:, None]
    c = np.arange(qw)[None, :]
    d = c - delta - p
    return ((d >= 0) & (d < win)).astype(np.float32)


def _band_images():
    import ml_dtypes
    bs = np.concatenate([_pattern(d, 256, WIN_S) for d in (-256, -128, 0, 128)],
                        axis=1)
    bl = np.concatenate(
        [np.concatenate([_pattern(da, 512, WIN_L), _pattern(da + 128, 512, WIN_L)],
                        axis=1)
         for da in (-1024, -768, 0, 256)], axis=1)
    return (np.ascontiguousarray(bs.astype(ml_dtypes.bfloat16)),
            np.ascontiguousarray(bl.astype(ml_dtypes.bfloat16)))


def make_in_maps(x, Wqk_short, Wv_short, Wqk_long, Wv_long, Wproj):
    """Host-side sharding: per-core input dict for core c = 4*b + g."""
    x = np.ascontiguousarray(np.asarray(x, dtype=np.float32))
    Wqk_short = np.asarray(Wqk_short, dtype=np.float32)
    Wv_short = np.asarray(Wv_short, dtype=np.float32)
    Wqk_long = np.asarray(Wqk_long, dtype=np.float32)
    Wv_long = np.asarray(Wv_long, dtype=np.float32)
    Wproj = np.asarray(Wproj, dtype=np.float32)
    assert x.shape == (B, T, C)

    xts = [np.ascontiguousarray(x[b].T) for b in range(B)]
    band_s, band_l = _band_images()
    in_maps = []
    for c in range(N_CORES):
        b, g = divmod(c, 4)
        wsqk = np.ascontiguousarray(np.concatenate(
            [Wqk_short[:, g * 64:(g + 1) * 64],
             Wqk_short[:, 256 + g * 64: 256 + (g + 1) * 64]], axis=1))
        wql = np.ascontiguousarray(Wqk_long[:, g * 256:(g + 1) * 256])
        wkl = np.ascontiguousarray(Wqk_long[:, 1024 + g * 256: 1024 + (g + 1) * 256])
        wv = np.ascontiguousarray(np.concatenate(
            [Wv_short[:, g * 128:(g + 1) * 128],
             Wv_long[:, g * 128:(g + 1) * 128]], axis=1))
        wp = np.ascontiguousarray(np.concatenate(
            [Wproj[g * 128:(g + 1) * 128, :],
             Wproj[512 + g * 128: 512 + (g + 1) * 128, :]], axis=0))
        in_maps.append({
            "xt": xts[b], "wsqk": wsqk, "wql": wql, "wkl": wkl, "wv": wv, "wp": wp,
            "band_s": band_s, "band_l": band_l,
        })
    return in_maps


def gather(results) -> np.ndarray:
    out = np.empty((B, T, C), dtype=np.float32)
    for b in range(B):
        acc = np.zeros((T, C), dtype=np.float32)
        for g in range(4):
            acc += np.asarray(results[4 * b + g]["out"], dtype=np.float32)
        out[b] = acc
    return out


def kernel(x, Wqk_short, Wv_short, Wqk_long, Wv_long, Wproj, **run_kwargs):
    nc = _get_program()
    in_maps = make_in_maps(x, Wqk_short, Wv_short, Wqk_long, Wv_long, Wproj)
    res = run_bass_kernel_spmd(nc, in_maps, core_ids=list(range(N_CORES)), **run_kwargs)
    out = gather(res.results)
    if run_kwargs:
        kernel.last_results = res
    return out


# revision 28
# speedup vs baseline: 1.8849x; 1.1417x over previous
"""Trainium2 Bass kernel for a two-window sparse causal self-attention block.

Model (B=2, T=2048, C=1024):
  - 8 "short" heads: d_qk=32,  window 256
  - 8 "long"  heads: d_qk=128, window 1024
  - value/output head dim 64, output projection C x C.

Sharding (8 cores): data-parallel over batch (2) x head-parallel over head
groups (4). Core c = 4*b + g handles batch b and heads {2g, 2g+1} of both the
short and long sets. Each core computes its 4 heads' attention plus the
corresponding 256 rows of Wproj, producing a partial [T, C] output (bf16); the
host sums the 4 partials per batch element in fp32.

v2 design notes (vs the first working version):
  - the tensor engine is kept continuously busy: per query group the
    (scores -> exp -> mask -> AV) chains of all heads are software-pipelined
    with an AV lag of D units, so the PE never sits in a cross-engine wait
    and its DVFS clock can ramp to 2.4 GHz.
  - long heads first, shorts after, so psum yh banks ring-recycle cleanly.
  - short heads use 256-wide query sub-blocks (window 256): computed score
    area per head drops from T*768 to T*512; N=256 keeps f32r at full rate.
  - exp outputs (softmax weights) and v tiles are bf16: AV matmuls run at
    1 cyc/row, DVE mask multiplies run in 2x packed mode, out DMA halves.
  - band masks are applied with ONE tensor_tensor per score strip using
    host-precomputed static 0/1 images concatenated per kb-offset; fully
    valid long pairs skip masking entirely.
  - softmax normalization: per-head reciprocal_approx_fast on the sums row,
    rank-1 matmul broadcast, one psum*psum multiply into the yts tile.
  - the output projection of group g is emitted interleaved into group g+1's
    score stream so it never stalls the group boundary.
"""

import contextlib
import math
from collections import deque


def _nullctx():
    return contextlib.nullcontext()

import numpy as np

import concourse.bass as bass
import concourse.mybir as mybir
import concourse.tile as tile
from concourse.bass_utils import run_bass_kernel_spmd

F32 = mybir.dt.float32
F32R = mybir.dt.float32r
BF16 = mybir.dt.bfloat16

B, T, C = 2, 2048, 1024
HS, DS = 8, 32
HL, DL = 8, 128
HD = 64
WIN_S, WIN_L = 256, 1024
NT = T // 128    # 16 t-blocks
NCB = C // 128   # 8 c-blocks
NG = T // 512    # 4 query groups
VW = HD + 1      # v columns + ones column for softmax sums
N_CORES = 8
AV_LAG = 4       # units between a strip's score matmuls and its AV matmuls


def _split_waits(nc: bass.Bass) -> int:
    """Walrus in this env accepts at most 1 sync wait per instruction.
    Hoist extra waits onto same-engine InstNoOp instructions placed just
    before the owning instruction (same-engine program order preserves the
    blocking semantics)."""
    import bass_rust

    n_added = 0
    for f in nc.m.functions:
        for bb in f.blocks:
            insts = bb.instructions
            if not any(inst.sync_info and len(inst.sync_info.on_wait) > 1
                       for inst in insts):
                continue
            new = []
            for inst in insts:
                si = inst.sync_info
                waits = list(si.on_wait) if si else []
                if len(waits) > 1:
                    for i, w in enumerate(waits[:-1]):
                        nop = mybir.InstNoOp(
                            name=f"{inst.name}_hw{i}",
                            sync_info=bass_rust.SyncInfo(on_wait=[w], on_update=[]),
                            bass_nofuse=True,
                            engine=inst.engine,
                        )
                        new.append(nop)
                        n_added += 1
                    inst.sync_info = bass_rust.SyncInfo(
                        on_wait=waits[-1:], on_update=list(si.on_update))
                new.append(inst)
            bb.instructions = new
    return n_added


def _patch_tile_drain():
    """This walrus build rejects >1 sync wait on the TileContext tail drain
    ("Too many sync wait commands"). Re-emit the drain's waits as individual
    wait_ge instructions on the sync engine."""
    import bass_rust
    from concourse.tile import ScopedClock, TileContext

    def _drain_and_barrier(self, tick_clock, wait_clock):
        nc = self.nc
        drain_inst = nc.sync.drain()
        wait_clock.add_sem_waits(
            drain_inst.ins, ScopedClock({None: tick_clock.global_clock})
        )
        si = drain_inst.ins.sync_info
        waits = list(si.on_wait) if si is not None else []
        if len(waits) > 1:
            drain_inst.ins.sync_info = bass_rust.SyncInfo(on_wait=[], on_update=[])
            sems = {h.name: h for h in self.sems.allocated().values()}
            for w in waits:
                nc.sync.wait_ge(sems[w.ant_name], w.wait_value)
        nc.all_engine_barrier()
        popped = nc._tile_sem_poison_stack.pop()
        assert popped is self._sem_poison
        nc.clear_and_free_semaphores(list(self.sems.allocated().values()))
        nc.all_engine_barrier()

    TileContext._drain_and_barrier = _drain_and_barrier


_patch_tile_drain()

# long pair images present in band_l, keyed by pair index j = (delta_a+1024)/256
_LONG_JMAP = {0: 0, 1: 1, 4: 2, 5: 3}


def _build_program() -> bass.Bass:
    nc = bass.Bass()

    xt_d = nc.dram_tensor("xt", [C, T], BF16, kind="ExternalInput")
    wsqk_d = nc.dram_tensor("wsqk", [C, 128], BF16, kind="ExternalInput")
    wql_d = nc.dram_tensor("wql", [C, 256], BF16, kind="ExternalInput")
    wkl_d = nc.dram_tensor("wkl", [C, 256], BF16, kind="ExternalInput")
    wv_d = nc.dram_tensor("wv", [C, 256], BF16, kind="ExternalInput")
    wp_d = nc.dram_tensor("wp", [256, C], BF16, kind="ExternalInput")
    bs_d = nc.dram_tensor("band_s", [128, 1024], BF16, kind="ExternalInput")
    bl_d = nc.dram_tensor("band_l", [128, 4096], BF16, kind="ExternalInput")
    out_d = nc.dram_tensor("out", [T, C], BF16, kind="ExternalOutput")

    scale_s = 1.0 / math.sqrt(DS)
    scale_l = 1.0 / math.sqrt(DL)

    with tile.TileContext(nc) as tc:
        with (
            tc.tile_pool(name="const", bufs=1) as const,
            tc.tile_pool(name="qkp", bufs=1) as qkp,
            tc.tile_pool(name="vp", bufs=1) as vp,
            tc.tile_pool(name="stp", bufs=2, space="PSUM") as stp,
            tc.tile_pool(name="ptp", bufs=6) as ptp,
        ):
            # ---- stage A weights (f32r views of the fp32 DRAM data) ----
            # DMA order matters for startup: wsqk + xt chunk 0 first so the
            # first projection matmul can start ~8us in; the remaining
            # weights stream in while chunk-0 compute runs.
            wsqk = const.tile([128, NCB, 128], BF16, tag="wsqk", name="wsqk")
            nc.sync.dma_start(wsqk[:], wsqk_d[:, :].rearrange("(cb p) d -> p cb d", p=128))
            wql = const.tile([128, NCB, 256], BF16, tag="wql", name="wql")
            wkl = const.tile([128, NCB, 256], BF16, tag="wkl", name="wkl")
            wv = const.tile([128, NCB, 256], BF16, tag="wv", name="wv")

            # ---- projection outputs (persist across both stages) ----
            qts = qkp.tile([64, T], F32R, tag="qts", name="qts")
            kts = qkp.tile([64, T], F32R, tag="kts", name="kts")
            qtl = [qkp.tile([128, T], F32R, tag=f"qtl{h}", name=f"qtl{h}") for h in range(2)]
            ktl = [qkp.tile([128, T], F32R, tag=f"ktl{h}", name=f"ktl{h}") for h in range(2)]
            # v for all 4 heads (s0, s1, l0, l1), bf16, ones col at index HD
            vt = vp.tile([128, 4, NT, VW], BF16, tag="vt", name="vt")
            for i in range(4):
                nc.vector.memset(vt[:, i, :, HD], 1.0)

            # ================= stage A: projections =================
            with (
                tc.tile_pool(name="xtp", bufs=1) as xtp,
                tc.tile_pool(name="vps", bufs=2, space="PSUM") as vps,
            ):
                xt = xtp.tile([128, NCB, T], BF16, tag="xt", name="xt")
                for tch in range(T // 512):
                    sl = slice(tch * 512, (tch + 1) * 512)
                    if tch == 0:
                        # stream chunk 0 per c-block: the first matmul of the
                        # first projection job only needs cb 0, so compute can
                        # start after ~0.75MB instead of 2.5MB
                        for cb in range(NCB):
                            nc.sync.dma_start(
                                xt[:, cb, sl],
                                xt_d[cb * 128:(cb + 1) * 128, sl])
                    else:
                        nc.sync.dma_start(
                            xt[:, :, sl],
                            xt_d[:, sl].rearrange("(cb p) t -> p cb t", p=128))
                    if tch == 0:
                        nc.sync.dma_start(wql[:], wql_d[:, :].rearrange("(cb p) d -> p cb d", p=128))
                        nc.sync.dma_start(wkl[:], wkl_d[:, :].rearrange("(cb p) d -> p cb d", p=128))
                        nc.sync.dma_start(wv[:], wv_d[:, :].rearrange("(cb p) d -> p cb d", p=128))

                proj_jobs = [(wsqk, None, None)]
                for h in range(2):
                    proj_jobs.append((wql, h, qtl[h]))
                    proj_jobs.append((wkl, h, ktl[h]))
                cp_alt = [0]

                def _acopy(dst_ap, src_ap, low=False):
                    eng = nc.scalar if cp_alt[0] % 2 == 0 else nc.vector
                    cp_alt[0] += 1
                    ctx = (nc.allow_low_precision(reason="bf16 v tiles") if low
                           else _nullctx())
                    with ctx:
                        if eng is nc.scalar:
                            eng.copy(dst_ap, src_ap)
                        else:
                            eng.tensor_copy(dst_ap, src_ap)

                for tch in range(T // 512):
                    sl = slice(tch * 512, (tch + 1) * 512)
                    for w, h, dst in proj_jobs:
                        ps = stp.tile([128, 1024], F32, tag="st", name="st")
                        for cb in range(NCB):
                            lhsT = w[:, cb, :] if h is None else w[:, cb, h * 128:(h + 1) * 128]
                            nc.tensor.matmul(
                                ps[:, 0:512], lhsT, xt[:, cb, sl],
                                start=(cb == 0), stop=(cb == NCB - 1),
                            )
                        if dst is None:
                            _acopy(qts[:, sl], ps[0:64, 0:512])
                            _acopy(kts[:, sl], ps[64:128, 0:512])
                        else:
                            _acopy(dst[:, sl], ps[:, 0:512])
                    for tb in range(4 * tch, 4 * tch + 4):
                        pv = vps.tile([128, 512], F32, tag="pv", name="pv")
                        for cb in range(NCB):
                            nc.tensor.matmul(
                                pv[:, 0:256], xt[:, cb, tb * 128:(tb + 1) * 128], wv[:, cb, :],
                                start=(cb == 0), stop=(cb == NCB - 1),
                            )
                        _acopy(vt[:, :, tb, 0:HD],
                               pv[:, 0:256].rearrange("p (i d) -> p i d", i=4),
                               low=True)

            # ============ stage B: attention + output projection ============
            with (
                tc.tile_pool(name="attnc", bufs=1) as attnc,
                tc.tile_pool(name="ytp", bufs=2) as ytp,
                tc.tile_pool(name="obp", bufs=4) as obp,
                tc.tile_pool(name="smallp", bufs=4) as smallp,
                tc.tile_pool(name="yhp", bufs=2, space="PSUM") as yhp,
                tc.tile_pool(name="onebank", bufs=2, space="PSUM") as onebank,
            ):
                wp0 = attnc.tile([128, C], BF16, tag="wp0", name="wp0")
                nc.sync.dma_start(wp0[:], wp_d[0:128, :])
                wp1 = attnc.tile([128, C], BF16, tag="wp1", name="wp1")
                nc.sync.dma_start(wp1[:], wp_d[128:256, :])
                band_s = attnc.tile([128, 1024], BF16, tag="band_s", name="band_s")
                nc.sync.dma_start(band_s[:], bs_d[:, :])
                band_l = attnc.tile([128, 4, 1024], BF16, tag="band_l", name="band_l")
                nc.sync.dma_start(band_l[:], bl_d[:, :].rearrange("p (j u) -> p j u", j=4))
                ones16 = attnc.tile([128, 64], BF16, tag="ones16", name="ones16")
                nc.vector.memset(ones16[:], 1.0)

                pend_wproj = []   # deferred output-projection emitters
                ob_alt = [0]      # rotates ob copies across scalar/vector
                msk_alt = [0]     # rotates mask multiplies across vector/gpsimd
                yv_alt = [0]      # rotates yv copies across scalar/vector

                def emit_wproj(yts_pair, q0):
                    ems = []
                    for sub in range(4):
                        for nh in range(2):
                            def em(sub=sub, nh=nh):
                                po = onebank.tile([128, 512], F32, tag="ob1", name="ob1")
                                ssl = (slice(None), slice(sub * 128, (sub + 1) * 128))
                                nc.tensor.matmul(po[:], yts_pair[0][ssl],
                                                 wp0[:, nh * 512:(nh + 1) * 512],
                                                 start=True, stop=False)
                                nc.tensor.matmul(po[:], yts_pair[1][ssl],
                                                 wp1[:, nh * 512:(nh + 1) * 512],
                                                 start=False, stop=True)
                                ob = obp.tile([128, 512], BF16, tag="ob", name="ob")
                                eng = nc.scalar if ob_alt[0] % 2 == 0 else nc.vector
                                ob_alt[0] += 1
                                with nc.allow_low_precision(reason="bf16 out"):
                                    if eng is nc.scalar:
                                        eng.copy(ob[:], po[:])
                                    else:
                                        eng.tensor_copy(ob[:], po[:])
                                qs = q0 + sub * 128
                                nc.sync.dma_start(
                                    out_d[qs:qs + 128, nh * 512:(nh + 1) * 512], ob[:])
                            ems.append(em)
                    return ems

                for qg in range(NG):
                    q0 = qg * 512
                    yts = [ytp.tile([128, 512], BF16, tag=f"yts{i}", name=f"yts{i}")
                           for i in range(2)]
                    # per-head state: [yh tile, avs_emitted, avs_total]
                    hstate = {}

                    norm = {"s4": None, "recs": []}
                    HIDX = {("L", 0): 0, ("L", 1): 1, ("S", 0): 2, ("S", 1): 3}

                    def phase1(key, yh, dest, poff):
                        # extract sums row + values, freeing the yh psum bank
                        i = HIDX[key]
                        if norm["s4"] is None:
                            norm["s4"] = smallp.tile([97, 512], F32, tag="s4",
                                                     name="s4")
                        s4 = norm["s4"]
                        nc.vector.tensor_copy(s4[32 * i:32 * i + 1, :],
                                              yh[HD:HD + 1, :])
                        yv = smallp.tile([64, 512], F32, tag="yv", name="yv")
                        eng = nc.scalar if yv_alt[0] % 2 == 0 else nc.vector
                        yv_alt[0] += 1
                        if eng is nc.scalar:
                            eng.copy(yv[:], yh[0:HD, :])
                        else:
                            eng.tensor_copy(yv[:], yh[0:HD, :])
                        norm["recs"].append((i, yv, dest, poff))

                    def phase2_emitters():
                        # deferred into the next group's unit stream so the
                        # recip chain never blocks the tensor engine
                        nrm = dict(norm)
                        state = {}

                        def em_recip():
                            # 1/s via exp(-ln(s)) on the scalar engine: Ln and
                            # Exp share an activation table, and this keeps the
                            # 3.3us DVE reciprocal off the mask-critical vector
                            # engine entirely.
                            s4 = nrm["s4"]
                            u4 = smallp.tile([97, 512], F32, tag="u4", name="u4")
                            nc.scalar.activation(u4[:], s4[:],
                                                 mybir.ActivationFunctionType.Ln)
                            r16 = smallp.tile([97, 512], BF16, tag="r16", name="r16")
                            with nc.allow_low_precision(reason="bf16 recip"):
                                nc.scalar.activation(
                                    r16[:], u4[:],
                                    mybir.ActivationFunctionType.Exp, scale=-1.0)
                            # matmul base partitions are limited to {0,32,64}:
                            # relocate head 3's reciprocal row to partition 0
                            r3 = smallp.tile([1, 512], BF16, tag="r3", name="r3")
                            nc.scalar.copy(r3[:], r16[96:97, :])
                            state["r16"], state["r3"] = r16, r3

                        ems = []  # noqa: E306
                        for rec in nrm["recs"]:
                            def em_norm(rec=rec):
                                i, yv, dest, poff = rec
                                r16, r3 = state["r16"], state["r3"]
                                rb = onebank.tile([128, 512], F32, tag="ob1",
                                                  name="ob1")
                                rsrc = r3[0:1, :] if i == 3 else r16[32 * i:32 * i + 1, :]
                                osrc = ones16[0:1, 0:64] if i == 3 else ones16[32 * i:32 * i + 1, 0:64]
                                nc.tensor.matmul(rb[0:64, :], osrc, rsrc,
                                                 start=True, stop=True)
                                with nc.allow_low_precision(reason="f32r attn out"):
                                    nc.vector.tensor_mul(dest[poff:poff + 64, :],
                                                         yv[:], rb[0:64, :])
                            ems.append(em_norm)
                        return em_recip, ems

                    units = []
                    # ---- long heads, h0/h1 interleaved per kb-pair ----
                    kb_lo = max(0, (q0 - WIN_L) // 128)
                    kb_hi = (q0 + 384) // 128
                    kbs_l = list(range(kb_lo, kb_hi + 1))
                    pairs = [(kbs_l[j], kbs_l[j + 1]) for j in range(0, len(kbs_l), 2)]
                    for pi, pair in enumerate(pairs):
                        for h in range(2):
                            units.append(("L", h, pair, pi == 0, pi == len(pairs) - 1))
                    # ---- short heads, 256-wide sub-blocks ----
                    sq_kbs = []
                    for sq in range(2):
                        q0s = q0 + 256 * sq
                        lo = max(0, (q0s - WIN_S) // 128)
                        hi = (q0s + 128) // 128
                        sq_kbs.append(list(range(lo, hi + 1)))
                    for sq in range(2):
                        for h in range(2):
                            units.append(("S", h, sq, sq == 0, sq == 1))

                    def emit_scores(u):
                        kind = u[0]
                        if kind == "L":
                            _, h, pair, _, _ = u
                            st = stp.tile([128, 1024], F32, tag="st", name="st")
                            for jj, kb in enumerate(pair):
                                nc.tensor.matmul(
                                    st[:, jj * 512:(jj + 1) * 512],
                                    ktl[h][:, kb * 128:(kb + 1) * 128],
                                    qtl[h][:, q0:q0 + 512], start=True, stop=True)
                            pt = ptp.tile([128, 1024], BF16, tag="pt", name="pt")
                            with nc.allow_low_precision(reason="bf16 softmax wts"):
                                nc.scalar.activation(
                                    pt[:], st[:],
                                    mybir.ActivationFunctionType.Exp, scale=scale_l)
                            j = (pair[0] * 128 - q0 + 1024) // 256
                            if j in _LONG_JMAP:
                                eng = nc.vector if msk_alt[0] % 2 == 0 else nc.gpsimd
                                msk_alt[0] += 1
                                eng.tensor_tensor(
                                    out=pt[:], in0=pt[:],
                                    in1=band_l[:, _LONG_JMAP[j], :],
                                    op=mybir.AluOpType.mult)
                            return pt
                        else:
                            _, h, sq, _, _ = u
                            q0s = q0 + 256 * sq
                            kbs = sq_kbs[sq]
                            wdt = 256 * len(kbs)
                            st = stp.tile([128, 1024], F32, tag="st", name="st")
                            for jj, kb in enumerate(kbs):
                                nc.tensor.matmul(
                                    st[:, jj * 256:(jj + 1) * 256],
                                    kts[32 * h:32 * h + 32, kb * 128:(kb + 1) * 128],
                                    qts[32 * h:32 * h + 32, q0s:q0s + 256],
                                    start=True, stop=True)
                            pt = ptp.tile([128, 1024], BF16, tag="pt", name="pt")
                            with nc.allow_low_precision(reason="bf16 softmax wts"):
                                nc.scalar.activation(
                                    pt[:, 0:wdt], st[:, 0:wdt],
                                    mybir.ActivationFunctionType.Exp, scale=scale_s)
                            eng = nc.vector if msk_alt[0] % 2 == 0 else nc.gpsimd
                            msk_alt[0] += 1
                            eng.tensor_tensor(
                                out=pt[:, 0:wdt], in0=pt[:, 0:wdt],
                                in1=band_s[:, 1024 - wdt:1024],
                                op=mybir.AluOpType.mult)
                            return pt

                    def emit_av(u, pt):
                        kind = u[0]
                        if kind == "L":
                            _, h, pair, first, last = u
                            key = ("L", h)
                            if key not in hstate:
                                hstate[key] = yhp.tile([VW, 512], F32, tag="yh",
                                                       name="yh")
                            yh = hstate[key]
                            for jj, kb in enumerate(pair):
                                nc.tensor.matmul(
                                    yh[:], vt[:, 2 + h, kb, :],
                                    pt[:, jj * 512:(jj + 1) * 512],
                                    start=(first and jj == 0),
                                    stop=(last and jj == len(pair) - 1))
                            if last:
                                phase1(("L", h), yh, yts[1], 64 * h)
                        else:
                            _, h, sq, first, last = u
                            key = ("S", h)
                            if key not in hstate:
                                hstate[key] = yhp.tile([VW, 512], F32, tag="yh",
                                                       name="yh")
                            yh = hstate[key]
                            kbs = sq_kbs[sq]
                            for jj, kb in enumerate(kbs):
                                nc.tensor.matmul(
                                    yh[:, sq * 256:(sq + 1) * 256],
                                    vt[:, h, kb, :],
                                    pt[:, jj * 256:(jj + 1) * 256],
                                    start=(first and jj == 0),
                                    stop=(last and jj == len(kbs) - 1))
                            if last:
                                phase1(("S", h), yh, yts[0], 64 * h)

                    pend_av = deque()
                    for u in units:
                        pt = emit_scores(u)
                        pend_av.append((u, pt))
                        if pend_wproj:
                            pend_wproj.pop(0)()
                        if len(pend_av) > AV_LAG:
                            emit_av(*pend_av.popleft())
                    while pend_av:
                        emit_av(*pend_av.popleft())
                    while pend_wproj:
                        pend_wproj.pop(0)()
                    recip_fn, tail_ems = phase2_emitters()
                    recip_fn()
                    pend_wproj = tail_ems + emit_wproj(yts, q0)
                while pend_wproj:
                    pend_wproj.pop(0)()

    return nc


_PROGRAM = None


def _get_program() -> bass.Bass:
    global _PROGRAM
    if _PROGRAM is None:
        _PROGRAM = _build_program()
        _split_waits(_PROGRAM)
    return _PROGRAM


def _pattern(delta: int, qw: int, win: int) -> np.ndarray:
    """[128, qw] 0/1 validity image for a key block at offset delta from the
    query block: cell (p, c) valid iff 0 <= (c - delta - p) < win."""
    p = np.arange(128)[:, None]
    c = np.arange(qw)[None, :]
    d = c - delta - p
    return ((d >= 0) & (d < win)).astype(np.float32)


def _band_images():
    import ml_dtypes
    bs = np.concatenate([_pattern(d, 256, WIN_S) for d in (-256, -128, 0, 128)],
                        axis=1)
    bl = np.concatenate(
        [np.concatenate([_pattern(da, 512, WIN_L), _pattern(da + 128, 512, WIN_L)],
                        axis=1)
         for da in (-1024, -768, 0, 256)], axis=1)
    return (np.ascontiguousarray(bs.astype(ml_dtypes.bfloat16)),
            np.ascontiguousarray(bl.astype(ml_dtypes.bfloat16)))


def make_in_maps(x, Wqk_short, Wv_short, Wqk_long, Wv_long, Wproj):
    """Host-side sharding: per-core input dict for core c = 4*b + g."""
    import ml_dtypes
    bf16 = ml_dtypes.bfloat16
    x = np.asarray(x, dtype=np.float32)
    Wqk_short = np.asarray(Wqk_short, dtype=np.float32).astype(bf16)
    Wv_short = np.asarray(Wv_short, dtype=np.float32).astype(bf16)
    Wqk_long = np.asarray(Wqk_long, dtype=np.float32).astype(bf16)
    Wv_long = np.asarray(Wv_long, dtype=np.float32).astype(bf16)
    Wproj = np.asarray(Wproj, dtype=np.float32).astype(bf16)
    assert x.shape == (B, T, C)

    xts = [np.ascontiguousarray(x[b].T.astype(bf16)) for b in range(B)]
    band_s, band_l = _band_images()
    in_maps = []
    for c in range(N_CORES):
        b, g = divmod(c, 4)
        wsqk = np.ascontiguousarray(np.concatenate(
            [Wqk_short[:, g * 64:(g + 1) * 64],
             Wqk_short[:, 256 + g * 64: 256 + (g + 1) * 64]], axis=1))
        wql = np.ascontiguousarray(Wqk_long[:, g * 256:(g + 1) * 256])
        wkl = np.ascontiguousarray(Wqk_long[:, 1024 + g * 256: 1024 + (g + 1) * 256])
        wv = np.ascontiguousarray(np.concatenate(
            [Wv_short[:, g * 128:(g + 1) * 128],
             Wv_long[:, g * 128:(g + 1) * 128]], axis=1))
        wp = np.ascontiguousarray(np.concatenate(
            [Wproj[g * 128:(g + 1) * 128, :],
             Wproj[512 + g * 128: 512 + (g + 1) * 128, :]], axis=0))
        in_maps.append({
            "xt": xts[b], "wsqk": wsqk, "wql": wql, "wkl": wkl, "wv": wv, "wp": wp,
            "band_s": band_s, "band_l": band_l,
        })
    return in_maps


def gather(results) -> np.ndarray:
    out = np.empty((B, T, C), dtype=np.float32)
    for b in range(B):
        acc = np.zeros((T, C), dtype=np.float32)
        for g in range(4):
            acc += np.asarray(results[4 * b + g]["out"], dtype=np.float32)
        out[b] = acc
    return out


def kernel(x, Wqk_short, Wv_short, Wqk_long, Wv_long, Wproj, **run_kwargs):
    nc = _get_program()
    in_maps = make_in_maps(x, Wqk_short, Wv_short, Wqk_long, Wv_long, Wproj)
    res = run_bass_kernel_spmd(nc, in_maps, core_ids=list(range(N_CORES)), **run_kwargs)
    out = gather(res.results)
    if run_kwargs:
        kernel.last_results = res
    return out
